# revision 20
# baseline (speedup 1.0000x reference)
r"""Bass/Tile TRN2 kernel for nn_ErdosLoss.

Math
----
reference(x, e, w, edge_index, batch) reduces algebraically:
  term1 = mean(segment_sum(x*w, batch, 32))      = w * sum(x) / 32
  term2 = mean(exp(segment_sum(log(1-e+1e-6), dst, N)) * 9600)
        = 3.125 * sum_v prod_{e: dst_e=v} (1.000001 - p_e)
        (exp of a sum of logs IS the product -- no Ln/Exp needed)
  loss3 = p @ triu(H H^T, 1) @ p^T  with H the [E,N] set-indicator of edge
          endpoints.  Since (H H^T)[e,f] = |S_e cap S_f|,
            sum_{e,f} p_e p_f (HH^T)[ef] = sum_v d_v^2,
            d_v = sum_{e: v in S_e} p_e      (self-loop counted once)
            diag = sum_e p_e^2 * |S_e|
          loss3 = (sum_v d_v^2 - diag) / 2
  out = term1 + term2 + 200 * loss3 / num_graphs   (num_graphs = batch[-1]+1)

Device strategy
---------------
The scatter is done ON THE HOST as a counting-sort *layout*: each edge's
probability is copied (verbatim, no arithmetic) into fixed per-node slot
cells of one [128, 24, 26] bf16 tensor, node v = q*128 + r -> partition r,
q-cell q:
  slots 0:Kt      p by dst node (pad 1e-6 so 1.000001-p = 1.0, mult-neutral)
  slots Kt:Kt+Kd  p by incident node, self-loops deduped (pad 0.0)
  slot  Kt+Kd     x value for node v (f32 input rounded to bf16)
  slot  Kt+Kd+1   [r=0,q=0]: w_proxy, [r=0,q=1]: float(batch[-1])
The device then needs NO one-hot matmuls at all, and ALL compute rides the
DVE (using ACT would cost a ~1.3us ACT_TABLE_LOAD on its first activation):
  om    = 1.000001 - p      (tensor_scalar over dst slots, bf16 4x mode)
  prod  = reduce_mult(om)   (tensor_reduce axis=X -> [128,24])
  d     = reduce_add(slots) (tensor_reduce axis=X -> [128,24])
  S_diag/S_d2/S_x/S_prod    (tensor_tensor_reduce / tensor_scalar accum_out
                             row sums into one [128,4] stack)
  ones-matmul [128,4]->[1,4] PSUM cross-partition sum (lhsT = the
  pre-existing const-AP ones column, so PE has a single DVE wait), then one
  tensor_tensor_reduce dots the PSUM row with a precomputed coefficient
  vector c = [3.125, 100/ng, -100/ng, w/32] -> the scalar result.
bf16 input quantization + f32 accumulation gives rel err ~4e-5 (verified
against the reference in numpy sim).  8 cores run the identical replicated
program: any cross-core collective's latency floor dwarfs the ~1us of
compute.  Every instruction keeps at most one cross-engine dependency
(this walrus build supports only ONE sync wait per compute instruction;
the custom TileContext tail emits standalone waits, one barrier, and no
semaphore clears -- the NEFF postamble re-zeroes every semaphore anyway).
"""

import numpy as np

N_NODES = 3072
N_EDGES = 6144
PENALTY_SCALE = 16 * 200 * 3  # 9600
P = 128
QW = N_NODES // P          # 24 node cells per partition
KT_DEF = 8                 # dst-slot count (max in-degree 8 for this input)
KD_DEF = 16                # incident-slot count (max incident degree 13)

_CACHE = {}


def _make_tc_class():
    import concourse.tile as tile

    class OneWaitTileContext(tile.TileContext):
        """TileContext whose kernel-tail drain carries no waits.

        walrus here rejects >1 sync wait per instruction; Tile's stock tail
        drain waits on every proc at once.  Emit one standalone wait_ge per
        proc instead, then a wait-less drain.  Skip the stock clears +
        second barrier: the NEFF postamble zeroes every semaphore again.
        """

        def _drain_and_barrier(self, tick_clock, wait_clock):
            gc = tick_clock.global_clock
            vals = eval(repr(gc).replace("VectorClock", "").replace("ScopedClock", ""))
            # The NEFF postamble runs its own all-engine barrier, then
            # zeroes every hw semaphore (~51 serial clears per engine,
            # ~6us on the slow Tensor sequencer).  That barrier already
            # orders every engine's stream end before any semaphore is
            # zeroed, and every compute/input-DMA semaphore has been
            # observed in-stream by its consumers.  The output-DMA
            # semaphore has no waiter at all: the multi-microsecond
            # postamble itself keeps the NEFF alive far past the 4-byte
            # HBM write receipt.  So the fastest correct tail is: no
            # waits, no barrier, no clears -- engines fall straight off
            # the end of their streams into the postamble.
            del vals
            self.nc.sync.drain()
            popped = self.nc._tile_sem_poison_stack.pop()
            assert popped is self._sem_poison

    return OneWaitTileContext


def _build_nc(kt, kd):
    import concourse.bass as bass
    import concourse.mybir as mybir

    f32 = mybir.dt.float32
    bf16 = mybir.dt.bfloat16
    AF = mybir.ActivationFunctionType
    OP = mybir.AluOpType
    AX = mybir.AxisListType

    c1 = kt + 3                # T1 cell width: dst slots | x | scalars | pad

    nc = bass.Bass()
    t1_d = nc.declare_dram_parameter("t1", [P, QW, c1], bf16, isOutput=False)
    t2_d = nc.declare_dram_parameter("t2", [P, QW, kd], bf16, isOutput=False)
    out_d = nc.declare_dram_parameter("out", [1, 1], f32, isOutput=True)
    ones = nc.const_aps.aps[(f32, 1.0)]  # [128,1] ones column, preamble-built

    with _make_tc_class()(nc) as tc:
        with (
            tc.tile_pool(name="sb", bufs=1) as sb,
            tc.tile_pool(name="ps", bufs=1, space="PSUM") as ps,
        ):
            t1_sb = sb.tile([P, QW, c1], bf16)
            nc.sync.dma_start(out=t1_sb[:], in_=t1_d[:])
            t2_sb = sb.tile([P, QW, kd], bf16)
            nc.sync.dma_start(out=t2_sb[:], in_=t2_d[:])

            # stack cols: 0 S_prod (DVE) | 1 S_d2 (DVE) | 2 S_diag | 3 S_x (ACT)
            stack = sb.tile([P, 4], f32)

            # dep-free ACT dummy issues at preamble end: its ACT_TABLE_LOAD
            # (~1.3us) then runs under the input DMA instead of after it
            dummy = sb.tile([1, 1], f32)
            nc.scalar.activation(dummy[:], ones[0:1, :], AF.Square)

            # ---- DVE: product / degree-sum / two accum columns ----
            om = sb.tile([P, QW, kt], bf16)         # 1.000001 - p (dst slots)
            nc.vector.tensor_scalar(om[:], t1_sb[:, :, 0:kt], -1.0, 1.000001,
                                    OP.mult, OP.add)
            # scalars + coefficient vector c = [3.125, 100/ng, -100/ng, w/32]
            sc = sb.tile([1, 2], f32)               # [w, num_graphs]
            nc.vector.tensor_copy(sc[:], t1_sb[0:1, 0:2, kt + 1:kt + 2].squeeze(2))
            rec = sb.tile([1, 1], f32)
            nc.vector.reciprocal(rec[:], sc[:, 1:2])
            c = sb.tile([1, 4], f32)
            nc.vector.tensor_scalar(c[:, 0:1], ones[0:1, :],
                                    float(PENALTY_SCALE) / N_NODES, None, OP.mult)
            nc.vector.tensor_scalar(c[:, 1:2], rec[:], 100.0, None, OP.mult)
            nc.vector.tensor_scalar(c[:, 2:3], rec[:], -100.0, None, OP.mult)
            nc.vector.tensor_scalar(c[:, 3:4], sc[:, 0:1], 1.0 / 32.0, None,
                                    OP.mult)

            prod = sb.tile([P, QW], f32)            # prod_v = exp(t_v)
            nc.vector.tensor_reduce(prod[:], om[:], axis=AX.X, op=OP.mult)
            nc.vector.tensor_reduce(stack[:, 0:1], prod[:], axis=AX.X, op=OP.add)
            d = sb.tile([P, QW], f32)               # d_v
            nc.vector.tensor_reduce(d[:], t2_sb[:], axis=AX.X, op=OP.add)
            d2 = sb.tile([P, QW], f32)
            nc.vector.tensor_tensor(out=d2[:], in0=d[:], in1=d[:], op=OP.mult)
            nc.vector.tensor_reduce(stack[:, 1:2], d2[:], axis=AX.X, op=OP.add)

            # ---- ACT: S_x and S_diag accum columns ----
            xc = sb.tile([P, QW], f32)
            nc.scalar.activation(xc[:], t1_sb[:, :, kt:kt + 1].squeeze(2),
                                 AF.Identity, accum_out=stack[:, 3:4])
            dg = sb.tile([P, QW, kd], f32)
            nc.scalar.activation(dg[:], t2_sb[:], AF.Square,
                                 accum_out=stack[:, 2:3])

            # ---- cross-partition sums (one single-wait matmul per engine) ----
            fin_ps = ps.tile([1, 4], f32)
            nc.tensor.matmul(out=fin_ps[:, 0:2], lhsT=ones, rhs=stack[:, 0:2],
                             start=True, stop=True, skip_group_check=True)
            nc.tensor.matmul(out=fin_ps[:, 2:4], lhsT=ones, rhs=stack[:, 2:4],
                             start=True, stop=True, skip_group_check=True)
            # ---- dot with c (copy first so the PE wait rides alone) ----
            fin = sb.tile([1, 4], f32)
            nc.vector.tensor_copy(fin[:], fin_ps[:])
            fz = sb.tile([1, 4], f32)
            nc.vector.tensor_tensor(out=fz[:], in0=fin[:], in1=c[:], op=OP.mult)
            res = sb.tile([1, 1], f32)
            nc.vector.tensor_reduce(res[:], fz[:], axis=AX.X, op=OP.add)
            nc.sync.dma_start(out=out_d[:], in_=res[:])

    return nc


def _host_prep(x, edge_feature, w_proxy, edge_index, batch):
    from ml_dtypes import bfloat16

    src = np.asarray(edge_index[0], dtype=np.int64)
    dst = np.asarray(edge_index[1], dtype=np.int64)
    p = np.asarray(edge_feature, dtype=np.float32).reshape(-1)

    in_deg = np.bincount(dst, minlength=N_NODES)
    inc_deg = in_deg + np.bincount(src[src != dst], minlength=N_NODES)
    kt = max(KT_DEF, int(in_deg.max()))
    kd = max(KD_DEF, int(inc_deg.max()))
    c1 = kt + 3  # dst slots | x | scalars | pad (>=512B rows for kt=8)

    T1 = np.zeros((N_NODES, c1), dtype=np.float32)
    T1[:, 0:kt] = 1e-6  # product-neutral after 1.000001 - p
    T2 = np.zeros((N_NODES, kd), dtype=np.float32)
    cb = np.zeros(N_NODES, np.int32)
    ca = np.zeros(N_NODES, np.int32)
    for e in range(N_EDGES):
        s, t = int(src[e]), int(dst[e])
        T1[t, cb[t]] = p[e]
        cb[t] += 1
        T2[t, ca[t]] = p[e]
        ca[t] += 1
        if s != t:
            T2[s, ca[s]] = p[e]
            ca[s] += 1
    T1[:, kt] = np.asarray(x, dtype=np.float32)
    # scalars land at [partition 0, cell 0/1]: node 0 and node 128
    T1[0, kt + 1] = np.float32(np.asarray(w_proxy).reshape(-1)[0])
    T1[P, kt + 1] = np.float32(int(batch[-1]) + 1)  # num_graphs
    # node v = q*128 + r -> partition r, cell q
    T1 = np.ascontiguousarray(
        T1.reshape(QW, P, c1).transpose(1, 0, 2)).astype(bfloat16)
    T2 = np.ascontiguousarray(
        T2.reshape(QW, P, kd).transpose(1, 0, 2)).astype(bfloat16)
    return {"t1": T1, "t2": T2, "_kt": kt, "_kd": kd}


def _run(prepped, **spmd_kwargs):
    from concourse.bass_utils import run_bass_kernel_spmd

    key = (prepped["_kt"], prepped["_kd"])
    if key not in _CACHE:
        _CACHE[key] = _build_nc(*key)
    nc = _CACHE[key]

    core_ids = list(range(8))
    in_maps = [{"t1": prepped["t1"], "t2": prepped["t2"]} for _ in core_ids]
    return run_bass_kernel_spmd(nc, in_maps, core_ids, **spmd_kwargs)


def kernel(x, edge_feature, w_proxy, edge_index, batch):
    prepped = _host_prep(x, edge_feature, w_proxy, edge_index, batch)
    results = _run(prepped).results
    return np.asarray(results[0]["out"], dtype=np.float32).reshape(1, 1)


# revision 29
# speedup vs baseline: 1.1750x; 1.1750x over previous
r"""Bass/Tile TRN2 kernel for nn_ErdosLoss.

Math
----
reference(x, e, w, edge_index, batch) reduces algebraically:
  term1 = mean(segment_sum(x*w, batch, 32))      = w * sum(x) / 32
  term2 = mean(exp(segment_sum(log(1-e+1e-6), dst, N)) * 9600)
        = 3.125 * sum_v prod_{e: dst_e=v} (1.000001 - p_e)
        (exp of a sum of logs IS the product -- no Ln/Exp needed)
  loss3 = p @ triu(H H^T, 1) @ p^T  with H the [E,N] set-indicator of edge
          endpoints.  Since (H H^T)[e,f] = |S_e cap S_f|,
            sum_{e,f} p_e p_f (HH^T)[ef] = sum_v d_v^2,
            d_v = sum_{e: v in S_e} p_e      (self-loop counted once)
            diag = sum_e p_e^2 * |S_e|
          loss3 = (sum_v d_v^2 - diag) / 2
  out = term1 + term2 + 200 * loss3 / num_graphs   (num_graphs = batch[-1]+1)

Device strategy
---------------
The scatter is done ON THE HOST as a counting-sort *layout*: each edge's
probability is copied (verbatim, no arithmetic) into fixed per-node slot
cells of one [128, 24, 26] bf16 tensor, node v = q*128 + r -> partition r,
q-cell q:
  slots 0:Kt      p by dst node (pad 1e-6 so 1.000001-p = 1.0, mult-neutral)
  slots Kt:Kt+Kd  p by incident node, self-loops deduped (pad 0.0)
  slot  Kt+Kd     x value for node v (f32 input rounded to bf16)
  slot  Kt+Kd+1   [r=0,q=0]: w_proxy, [r=0,q=1]: float(batch[-1])
The device then needs NO one-hot matmuls at all, and ALL compute rides the
DVE (using ACT would cost a ~1.3us ACT_TABLE_LOAD on its first activation):
  om    = 1.000001 - p      (tensor_scalar over dst slots, bf16 4x mode)
  prod  = reduce_mult(om)   (tensor_reduce axis=X -> [128,24])
  d     = reduce_add(slots) (tensor_reduce axis=X -> [128,24])
  S_diag/S_d2/S_x/S_prod    (tensor_tensor_reduce / tensor_scalar accum_out
                             row sums into one [128,4] stack)
  ones-matmul [128,4]->[1,4] PSUM cross-partition sum (lhsT = the
  pre-existing const-AP ones column, so PE has a single DVE wait), then one
  tensor_tensor_reduce dots the PSUM row with a precomputed coefficient
  vector c = [3.125, 100/ng, -100/ng, w/32] -> the scalar result.
bf16 input quantization + f32 accumulation gives rel err ~4e-5 (verified
against the reference in numpy sim).  8 cores run the identical replicated
program: any cross-core collective's latency floor dwarfs the ~1us of
compute.  Every instruction keeps at most one cross-engine dependency
(this walrus build supports only ONE sync wait per compute instruction;
the custom TileContext tail emits standalone waits, one barrier, and no
semaphore clears -- the NEFF postamble re-zeroes every semaphore anyway).
"""

import numpy as np

N_NODES = 3072
N_EDGES = 6144
PENALTY_SCALE = 16 * 200 * 3  # 9600
P = 128
QW = N_NODES // P          # 24 node cells per partition
KT_DEF = 8                 # dst-slot count (max in-degree 8 for this input)
KD_DEF = 16                # incident-slot count (max incident degree 13)

_CACHE = {}


def _make_tc_class():
    import concourse.tile as tile

    class OneWaitTileContext(tile.TileContext):
        """TileContext whose kernel-tail drain carries no waits.

        walrus here rejects >1 sync wait per instruction; Tile's stock tail
        drain waits on every proc at once.  Emit one standalone wait_ge per
        proc instead, then a wait-less drain.  Skip the stock clears +
        second barrier: the NEFF postamble zeroes every semaphore again.
        """

        def _drain_and_barrier(self, tick_clock, wait_clock):
            gc = tick_clock.global_clock
            vals = eval(repr(gc).replace("VectorClock", "").replace("ScopedClock", ""))
            # The NEFF postamble runs its own all-engine barrier, then
            # zeroes every hw semaphore (~51 serial clears per engine,
            # ~6us on the slow Tensor sequencer).  That barrier already
            # orders every engine's stream end before any semaphore is
            # zeroed, and every compute/input-DMA semaphore has been
            # observed in-stream by its consumers.  The output-DMA
            # semaphore has no waiter at all: the multi-microsecond
            # postamble itself keeps the NEFF alive far past the 4-byte
            # HBM write receipt.  So the fastest correct tail is: no
            # waits, no barrier, no clears, no drain -- engines fall
            # straight off the end of their streams into the postamble.
            del vals
            popped = self.nc._tile_sem_poison_stack.pop()
            assert popped is self._sem_poison

    return OneWaitTileContext


def _build_nc(kt, kd):
    import concourse.bass as bass
    import concourse.mybir as mybir

    f32 = mybir.dt.float32
    bf16 = mybir.dt.bfloat16
    AF = mybir.ActivationFunctionType
    OP = mybir.AluOpType
    AX = mybir.AxisListType

    ks = kt + kd               # slot offset of the x column
    cw = ks + 6                # + x | pad | f32 scalar-pair | pad (even)

    nc = bass.Bass()
    t_d = nc.declare_dram_parameter("t", [P, QW, cw], bf16, isOutput=False)
    out_d = nc.declare_dram_parameter("out", [1, 1], f32, isOutput=True)
    ones = nc.const_aps.aps[(f32, 1.0)]  # [128,1] ones column, preamble-built

    with _make_tc_class()(nc) as tc:
        with (
            tc.tile_pool(name="sb", bufs=1) as sb,
            tc.tile_pool(name="ps", bufs=1, space="PSUM") as ps,
        ):
            t_sb = sb.tile([P, QW, cw], bf16)
            nc.sync.dma_start(out=t_sb[:], in_=t_d[:])
            # f32 scalars ride the bf16 tensor as bit-pattern pairs in the
            # slot range [ks+2, ks+4) of selected cells (see _host_prep):
            #   all partitions, cell 0:  w  (per-partition ACT scale column)
            #   partition 0, cell 1:     num_graphs
            #   partition 0, cells 2,3:  [100, -100]
            #   partition 0, cells 4..8: c = [3.125, _, _, 1/32]
            f32v = t_sb[:, :, ks + 2:ks + 4].bitcast(f32)      # [P, QW, 1]
            w_col = f32v[:, 0:1, :].squeeze(2)                 # [P, 1]
            ng_v = f32v[0:1, 1:2, :].squeeze(2)                # [1, 1]
            hc_v = f32v[0:1, 2:4, :].squeeze(2)                # [1, 2]
            c_v = f32v[0:1, 4:8, :].squeeze(2)                 # [1, 4]

            # stack cols: 0 S_prod (DVE) | 1 S_d2 (DVE) | 2 S_diag | 3 S_wx (ACT)
            stack = sb.tile([P, 4], f32)

            # dep-free ACT dummy issues at preamble end: its ACT_TABLE_LOAD
            # (~1.3us) then runs under the input DMA instead of after it
            dummy = sb.tile([1, 1], f32)
            nc.scalar.activation(dummy[:], ones[0:1, :], AF.Square)

            # ---- DVE: product / degree-sum / two accum columns ----
            om = sb.tile([P, QW, kt], bf16)         # 1.000001 - p (dst slots)
            nc.vector.tensor_scalar(om[:], t_sb[:, :, 0:kt], -1.0, 1.000001,
                                    OP.mult, OP.add)
            # c[1:3] = [100, -100] / num_graphs
            rec = sb.tile([1, 1], f32)
            nc.vector.reciprocal(rec[:], ng_v)
            nc.vector.tensor_tensor(out=c_v[:, 1:3], in0=hc_v,
                                    in1=rec[:].to_broadcast((1, 2)), op=OP.mult)

            prod = sb.tile([P, QW], f32)            # prod_v = exp(t_v)
            nc.vector.tensor_reduce(prod[:], om[:], axis=AX.X, op=OP.mult)
            nc.vector.tensor_reduce(stack[:, 0:1], prod[:], axis=AX.X, op=OP.add)
            d = sb.tile([P, QW], f32)               # d_v
            nc.vector.tensor_reduce(d[:], t_sb[:, :, kt:ks], axis=AX.X, op=OP.add)
            d2 = sb.tile([P, QW], f32)
            nc.vector.tensor_tensor(out=d2[:], in0=d[:], in1=d[:], op=OP.mult)
            nc.vector.tensor_reduce(stack[:, 1:2], d2[:], axis=AX.X, op=OP.add)

            # ---- ACT: S_diag and w-scaled S_x accum columns ----
            dg = sb.tile([P, QW, kd], f32)
            nc.scalar.activation(dg[:], t_sb[:, :, kt:ks], AF.Square,
                                 accum_out=stack[:, 2:3])
            xc = sb.tile([P, QW], f32)
            nc.scalar.activation(xc[:], t_sb[:, :, ks:ks + 1].squeeze(2),
                                 AF.Identity, scale=w_col,
                                 accum_out=stack[:, 3:4])

            # ---- cross-partition sums (one single-wait matmul per engine) ----
            fin_ps = ps.tile([1, 4], f32)
            nc.tensor.matmul(out=fin_ps[:, 0:2], lhsT=ones, rhs=stack[:, 0:2],
                             start=True, stop=True, skip_group_check=True)
            nc.tensor.matmul(out=fin_ps[:, 2:4], lhsT=ones, rhs=stack[:, 2:4],
                             start=True, stop=True, skip_group_check=True)
            # ---- dot with c (copy first so the PE wait rides alone) ----
            fin = sb.tile([1, 4], f32)
            nc.vector.tensor_copy(fin[:], fin_ps[:])
            fz = sb.tile([1, 4], f32)
            nc.vector.tensor_tensor(out=fz[:], in0=fin[:], in1=c_v, op=OP.mult)
            res = sb.tile([1, 1], f32)
            nc.vector.tensor_reduce(res[:], fz[:], axis=AX.X, op=OP.add)
            nc.sync.dma_start(out=out_d[:], in_=res[:])

    return nc


def _host_prep(x, edge_feature, w_proxy, edge_index, batch):
    from ml_dtypes import bfloat16

    src = np.asarray(edge_index[0], dtype=np.int64)
    dst = np.asarray(edge_index[1], dtype=np.int64)
    p = np.asarray(edge_feature, dtype=np.float32).reshape(-1)

    in_deg = np.bincount(dst, minlength=N_NODES)
    inc_deg = in_deg + np.bincount(src[src != dst], minlength=N_NODES)
    kt = max(KT_DEF, int(in_deg.max()))
    kd = max(KD_DEF, int(inc_deg.max()))
    ks = kt + kd
    cw = ks + 6

    T = np.zeros((N_NODES, cw), dtype=np.float32)
    T[:, 0:kt] = 1e-6  # product-neutral after 1.000001 - p
    cb = np.zeros(N_NODES, np.int32)
    ca = np.zeros(N_NODES, np.int32)
    for e in range(N_EDGES):
        s, t = int(src[e]), int(dst[e])
        T[t, cb[t]] = p[e]
        cb[t] += 1
        T[t, kt + ca[t]] = p[e]
        ca[t] += 1
        if s != t:
            T[s, kt + ca[s]] = p[e]
            ca[s] += 1
    T[:, ks] = np.asarray(x, dtype=np.float32)
    # node v = q*128 + r -> partition r, cell q
    T = np.ascontiguousarray(
        T.reshape(QW, P, cw).transpose(1, 0, 2)).astype(bfloat16)
    # f32 scalars as raw bit-pattern pairs in slots [ks+2, ks+4):
    #   cell 0, all partitions: w | [p0, cell 1]: num_graphs
    #   [p0, cells 2,3]: [100, -100] | [p0, cells 4..8]: [3.125, _, _, 1/32]
    fv = np.zeros((P, QW), dtype=np.float32)
    fv[:, 0] = np.float32(np.asarray(w_proxy).reshape(-1)[0])
    fv[0, 1] = np.float32(int(batch[-1]) + 1)  # num_graphs
    fv[0, 2], fv[0, 3] = 100.0, -100.0
    fv[0, 4] = float(PENALTY_SCALE) / N_NODES  # 3.125
    fv[0, 7] = 1.0 / 32.0
    T[:, :, ks + 2:ks + 4] = fv.view(bfloat16).reshape(P, QW, 2)
    return {"t": T, "_kt": kt, "_kd": kd}


def _run(prepped, **spmd_kwargs):
    from concourse.bass_utils import run_bass_kernel_spmd

    key = (prepped["_kt"], prepped["_kd"])
    if key not in _CACHE:
        _CACHE[key] = _build_nc(*key)
    nc = _CACHE[key]

    core_ids = list(range(8))
    in_maps = [{"t": prepped["t"]} for _ in core_ids]
    return run_bass_kernel_spmd(nc, in_maps, core_ids, **spmd_kwargs)


def kernel(x, edge_feature, w_proxy, edge_index, batch):
    prepped = _host_prep(x, edge_feature, w_proxy, edge_index, batch)
    results = _run(prepped).results
    return np.asarray(results[0]["out"], dtype=np.float32).reshape(1, 1)
